# revision 26
# baseline (speedup 1.0000x reference)
"""Trainium2 Bass kernel for nn_BD dense MLP (block-diagonal hidden layers).

Network: x[B,64] -> relu(x@W_in)[B,32] -> 4x relu(h@(mask*W_h))[B,32]
         -> h@(mask*W_out)[B,24]

Strategy (pure data parallel over 8 cores, B=1048576, R=131072 rows/core):
 - Host pre-transposes x into a feature-major grouped layout so NO on-device
   transpose is needed: SBUF partition p = g*32 + f (g = row-group 0..3,
   f = feature-within-half), free dim = fb*1024 + j (fb = feature half).
 - All matmuls bf16 (single PE pass): 128x128 block-diagonal stationaries
   kron(eye(4), W) process 4 row-groups per streamed column. L1 accumulates
   its two 32-feature halves into PSUM (contract=64).
 - The only mandatory elementwise work is the 6 PSUM(f32)->SBUF drains per
   slab (5 relu + 1 final copy); PSUM reads are capped at 4B/cycle/lane per
   engine, so they are split across ScalarE and VectorE to balance busy time
   (~3.4us/slab each vs PE ~3.0us).
 - L6 uses kron(eye(4), W_out) [128, 96] (24 real output cols, no padding);
   output leaves the chip feature-major in bf16 ([96, cols]); the host
   undoes the permutation and upcasts to f32.
 - Fully skewed software pipeline across 4096-row slabs.
"""

import sys

import numpy as np

if "/opt/trn_rl_repo" not in sys.path:
    sys.path.insert(0, "/opt/trn_rl_repo")

N_CORES = 8
B_FULL = 1048576
R = B_FULL // N_CORES  # rows per core
SLAB = 4096  # rows per pipeline iteration
# fractional drain split: DVE does relu4 cols [0:SPLIT), ACT does [SPLIT:1024)
SPLIT = 900


def build_nc(rows=R):
    """Build the single-core SPMD Bass graph."""
    import concourse.bass as bass  # noqa: F401
    import concourse.mybir as mybir
    from concourse import bacc, tile

    f32 = mybir.dt.float32
    bf16 = mybir.dt.bfloat16
    nc = bacc.Bacc(None)

    n_slabs = rows // SLAB
    xt_ext = nc.declare_dram_parameter("xt", [128, n_slabs * 2048], bf16, isOutput=False)
    # block-diagonal stationaries: L1 fb0, L1 fb1, L2..L5 (128 each), L6 (96)
    wbd_ext = nc.declare_dram_parameter("wbd", [128, 864], bf16, isOutput=False)
    out_ext = nc.declare_dram_parameter("out", [96, n_slabs * 1024], bf16, isOutput=True)

    x_r = xt_ext.rearrange("p (s c) -> s p c", c=2048)
    o_r = out_ext.rearrange("p (s c) -> s p c", c=1024)

    Relu = mybir.ActivationFunctionType.Relu

    with tile.TileContext(nc) as tc:
        with (
            tc.tile_pool(name="const", bufs=1) as cpool,
            tc.tile_pool(name="xin", bufs=6) as xpool,
            tc.tile_pool(name="h", bufs=12) as hpool,
            tc.tile_pool(name="ps", bufs=4, space="PSUM") as pspool,
            tc.tile_pool(name="ot", bufs=4) as otpool,
        ):
            wbd = cpool.tile([128, 864], bf16, tag="wbd")
            nc.sync.dma_start(wbd[:, :], wbd_ext[:, :])

            def wsl(i):  # 0..5 -> fb0, fb1, L2..L5 (128 cols); 6 -> L6 (96)
                return wbd[:, 128 * i : 128 * i + (96 if i == 6 else 128)]

            # Fully skewed software pipeline: step t advances slab t-k
            # through stage k. Stages: 0 load, 2 L1+relu1, 3..6 L2..L5+relu,
            # 7 L6+copy+store.
            st = [dict() for _ in range(n_slabs)]

            def ok(i):
                return 0 <= i < n_slabs

            # Stages are emitted OLDEST slab first (L6 .. L1) so the two
            # same-step PSUM buffer reuses (6 allocs on 4 bufs) pair the
            # earliest-issued drains with the latest matmul groups, and all
            # other reuse edges cross a step boundary with a full period of
            # slack.
            for t in range(n_slabs + 8):
                if ok(t):
                    x_sb = xpool.tile([128, 2048], bf16, tag="x")
                    nc.sync.dma_start(x_sb[:, :], x_r[t])
                    st[t]["x"] = x_sb

                if ok(t - 7):
                    s = t - 7
                    ps = pspool.tile([128, 1024], f32, tag="ps")
                    for hh in range(2):
                        nc.tensor.matmul(
                            ps[0:96, 512 * hh : 512 * hh + 512],
                            lhsT=wsl(6),
                            rhs=st[s]["h"][:, 512 * hh : 512 * hh + 512],
                            start=True,
                            stop=True,
                        )
                    ot = otpool.tile([96, 1024], bf16, tag="ot")
                    nc.vector.tensor_copy(ot[:, :], ps[0:96, :])
                    nc.sync.dma_start(o_r[s], ot[:, :])

                for l in range(3, -1, -1):
                    s = t - 3 - l
                    if ok(s):
                        ps = pspool.tile([128, 1024], f32, tag="ps")
                        for hh in range(2):
                            nc.tensor.matmul(
                                ps[:, 512 * hh : 512 * hh + 512],
                                lhsT=wsl(2 + l),
                                rhs=st[s]["h"][:, 512 * hh : 512 * hh + 512],
                                start=True,
                                stop=True,
                            )
                        h = hpool.tile([128, 1024], bf16, tag="h")
                        if l == 0:  # L2 relu on DVE
                            nc.vector.tensor_scalar_max(h[:, :], ps[:, :], 0.0)
                        elif l == 1:  # L3 relu on ACT
                            nc.scalar.activation(h[:, :], ps[:, :], Relu)
                        elif l == 2:  # L4 relu split DVE/ACT
                            nc.vector.tensor_scalar_max(
                                h[:, :SPLIT], ps[:, :SPLIT], 0.0
                            )
                            nc.scalar.activation(h[:, SPLIT:], ps[:, SPLIT:], Relu)
                        else:  # L5 relu on ACT
                            nc.scalar.activation(h[:, :], ps[:, :], Relu)
                        st[s]["h"] = h

                if ok(t - 2):
                    s = t - 2
                    x_sb = st[s]["x"]
                    ps = pspool.tile([128, 1024], f32, tag="ps")
                    for hh in range(2):
                        for fb in range(2):
                            nc.tensor.matmul(
                                ps[:, 512 * hh : 512 * hh + 512],
                                lhsT=wsl(fb),
                                rhs=x_sb[:, 1024 * fb + 512 * hh : 1024 * fb + 512 * hh + 512],
                                start=(fb == 0),
                                stop=(fb == 1),
                            )
                    h = hpool.tile([128, 1024], bf16, tag="h")
                    nc.scalar.activation(h[:, :], ps[:, :], Relu)
                    st[s]["h"] = h

    nc.compile()
    return nc


def prep_weights(input_weight, hidden_weights, output_weights):
    """Stationaries: L1 fb0/fb1 + L2..L5 [128,128] each, L6 [128,96]."""
    hid_filter = np.kron(np.eye(4, dtype=np.float32), np.ones((8, 8), np.float32))
    out_filter = np.kron(np.eye(8, dtype=np.float32), np.ones((4, 3), np.float32))
    whm = hid_filter[None] * np.asarray(hidden_weights, np.float32)  # [4,32,32]
    wom = out_filter * np.asarray(output_weights, np.float32)  # [32,24]
    w_in = np.asarray(input_weight, np.float32)  # [64,32]

    mats = []
    for fb in range(2):
        mats.append(np.kron(np.eye(4, dtype=np.float32), w_in[32 * fb : 32 * fb + 32]))
    for l in range(4):
        mats.append(np.kron(np.eye(4, dtype=np.float32), whm[l]))
    mats.append(np.kron(np.eye(4, dtype=np.float32), wom))  # [128, 96]
    return np.concatenate(mats, axis=1)  # [128, 864]


def to_bf16(a):
    import ml_dtypes

    return np.asarray(a, np.float32).astype(ml_dtypes.bfloat16)


def prep_x(x, n_cores=N_CORES):
    """[B, 64] f32 -> [n_cores, 128, rows/2] bf16 feature-major group layout.

    xt[core, g*32+f, s*2048 + fb*1024 + j] = x[core*R + s*4096 + g*1024 + j,
                                              fb*32 + f]
    """
    x = to_bf16(x)
    b = x.shape[0]
    rows = b // n_cores
    n_slabs = rows // SLAB
    v = x.reshape(n_cores, n_slabs, 4, 1024, 2, 32)  # (core, s, g, j, fb, f)
    v = v.transpose(0, 2, 5, 1, 4, 3)  # (core, g, f, s, fb, j)
    return np.ascontiguousarray(v).reshape(n_cores, 128, n_slabs * 2048)


def decode_out(a, rows):
    """[96, rows/4] bf16 feature-major -> [rows, 24] f32.

    a[g*24 + c, s*1024 + j] = out[s*4096 + g*1024 + j, c]
    """
    n_slabs = rows // SLAB
    a = np.asarray(a)
    v = a.reshape(4, 24, n_slabs, 1024)  # (g, c, s, j)
    v = v.transpose(2, 0, 3, 1)  # (s, g, j, c)
    return np.ascontiguousarray(v).reshape(rows, 24).astype(np.float32)


def kernel(x, input_weight, hidden_weights, output_weights):
    from concourse.bass_utils import run_bass_kernel_spmd

    xt = prep_x(x)
    wbd = to_bf16(prep_weights(input_weight, hidden_weights, output_weights))

    rows = x.shape[0] // N_CORES
    nc = build_nc(rows)
    in_maps = [{"xt": xt[i], "wbd": wbd} for i in range(N_CORES)]
    res = run_bass_kernel_spmd(nc, in_maps, core_ids=list(range(N_CORES)))
    outs = [decode_out(res.results[i]["out"], rows) for i in range(N_CORES)]
    return np.concatenate(outs, axis=0)


# revision 27
# speedup vs baseline: 1.0177x; 1.0177x over previous
"""Trainium2 Bass kernel for nn_BD dense MLP (block-diagonal hidden layers).

Network: x[B,64] -> relu(x@W_in)[B,32] -> 4x relu(h@(mask*W_h))[B,32]
         -> h@(mask*W_out)[B,24]

Strategy (pure data parallel over 8 cores, B=1048576, R=131072 rows/core):
 - Host pre-transposes x into a feature-major grouped layout so NO on-device
   transpose is needed: SBUF partition p = g*32 + f (g = row-group 0..3,
   f = feature-within-half), free dim = fb*1024 + j (fb = feature half).
 - All matmuls bf16 (single PE pass): 128x128 block-diagonal stationaries
   kron(eye(4), W) process 4 row-groups per streamed column. L1 accumulates
   its two 32-feature halves into PSUM (contract=64).
 - The only mandatory elementwise work is the 6 PSUM(f32)->SBUF drains per
   slab (5 relu + 1 final copy); PSUM reads are capped at 4B/cycle/lane per
   engine, so they are split across ScalarE and VectorE to balance busy time
   (~3.4us/slab each vs PE ~3.0us).
 - L6 uses kron(eye(4), W_out) [128, 96] (24 real output cols, no padding);
   output leaves the chip feature-major in bf16 ([96, cols]); the host
   undoes the permutation and upcasts to f32.
 - Fully skewed software pipeline across 4096-row slabs.
"""

import sys

import numpy as np

if "/opt/trn_rl_repo" not in sys.path:
    sys.path.insert(0, "/opt/trn_rl_repo")

N_CORES = 8
B_FULL = 1048576
R = B_FULL // N_CORES  # rows per core
SLAB = 4096  # rows per pipeline iteration
# fractional drain split: DVE does relu4 cols [0:SPLIT), ACT does [SPLIT:1024)
SPLIT = 876


def build_nc(rows=R):
    """Build the single-core SPMD Bass graph."""
    import concourse.bass as bass  # noqa: F401
    import concourse.mybir as mybir
    from concourse import bacc, tile

    f32 = mybir.dt.float32
    bf16 = mybir.dt.bfloat16
    nc = bacc.Bacc(None)

    n_slabs = rows // SLAB
    xt_ext = nc.declare_dram_parameter("xt", [128, n_slabs * 2048], bf16, isOutput=False)
    # block-diagonal stationaries: L1 fb0, L1 fb1, L2..L5 (128 each), L6 (96)
    wbd_ext = nc.declare_dram_parameter("wbd", [128, 864], bf16, isOutput=False)
    out_ext = nc.declare_dram_parameter("out", [96, n_slabs * 1024], bf16, isOutput=True)

    x_r = xt_ext.rearrange("p (s c) -> s p c", c=2048)
    o_r = out_ext.rearrange("p (s c) -> s p c", c=1024)

    Relu = mybir.ActivationFunctionType.Relu

    with tile.TileContext(nc) as tc:
        with (
            tc.tile_pool(name="const", bufs=1) as cpool,
            tc.tile_pool(name="xin", bufs=4) as xpool,
            tc.tile_pool(name="h", bufs=12) as hpool,
            tc.tile_pool(name="ps", bufs=4, space="PSUM") as pspool,
            tc.tile_pool(name="ot", bufs=4) as otpool,
        ):
            wbd = cpool.tile([128, 864], bf16, tag="wbd")
            nc.sync.dma_start(wbd[:, :], wbd_ext[:, :])

            def wsl(i):  # 0..5 -> fb0, fb1, L2..L5 (128 cols); 6 -> L6 (96)
                return wbd[:, 128 * i : 128 * i + (96 if i == 6 else 128)]

            # Fully skewed software pipeline: step t advances slab t-k
            # through stage k. Stages: 0 load, 2 L1+relu1, 3..6 L2..L5+relu,
            # 7 L6+copy+store.
            st = [dict() for _ in range(n_slabs)]

            def ok(i):
                return 0 <= i < n_slabs

            # Stages are emitted OLDEST slab first (L6 .. L1) so the two
            # same-step PSUM buffer reuses (6 allocs on 4 bufs) pair the
            # earliest-issued drains with the latest matmul groups, and all
            # other reuse edges cross a step boundary with a full period of
            # slack.
            for t in range(n_slabs + 8):
                if ok(t):
                    x_sb = xpool.tile([128, 2048], bf16, tag="x")
                    nc.sync.dma_start(x_sb[:, :], x_r[t])
                    st[t]["x"] = x_sb

                if ok(t - 7):
                    s = t - 7
                    ps = pspool.tile([128, 1024], f32, tag="ps")
                    for hh in range(2):
                        nc.tensor.matmul(
                            ps[0:96, 512 * hh : 512 * hh + 512],
                            lhsT=wsl(6),
                            rhs=st[s]["h"][:, 512 * hh : 512 * hh + 512],
                            start=True,
                            stop=True,
                        )
                    ot = otpool.tile([96, 1024], bf16, tag="ot")
                    nc.vector.tensor_copy(ot[:, :], ps[0:96, :])
                    nc.sync.dma_start(o_r[s], ot[:, :])

                for l in range(3, -1, -1):
                    s = t - 3 - l
                    if ok(s):
                        ps = pspool.tile([128, 1024], f32, tag="ps")
                        for hh in range(2):
                            nc.tensor.matmul(
                                ps[:, 512 * hh : 512 * hh + 512],
                                lhsT=wsl(2 + l),
                                rhs=st[s]["h"][:, 512 * hh : 512 * hh + 512],
                                start=True,
                                stop=True,
                            )
                        h = hpool.tile([128, 1024], bf16, tag="h")
                        if l == 0:  # L2 relu on DVE
                            nc.vector.tensor_scalar_max(h[:, :], ps[:, :], 0.0)
                        elif l == 1:  # L3 relu on ACT
                            nc.scalar.activation(h[:, :], ps[:, :], Relu)
                        elif l == 2:  # L4 relu split DVE/ACT
                            nc.vector.tensor_scalar_max(
                                h[:, :SPLIT], ps[:, :SPLIT], 0.0
                            )
                            nc.scalar.activation(h[:, SPLIT:], ps[:, SPLIT:], Relu)
                        else:  # L5 relu on ACT
                            nc.scalar.activation(h[:, :], ps[:, :], Relu)
                        st[s]["h"] = h

                if ok(t - 2):
                    s = t - 2
                    x_sb = st[s]["x"]
                    ps = pspool.tile([128, 1024], f32, tag="ps")
                    for hh in range(2):
                        for fb in range(2):
                            nc.tensor.matmul(
                                ps[:, 512 * hh : 512 * hh + 512],
                                lhsT=wsl(fb),
                                rhs=x_sb[:, 1024 * fb + 512 * hh : 1024 * fb + 512 * hh + 512],
                                start=(fb == 0),
                                stop=(fb == 1),
                            )
                    h = hpool.tile([128, 1024], bf16, tag="h")
                    nc.scalar.activation(h[:, :], ps[:, :], Relu)
                    st[s]["h"] = h

    nc.compile()
    return nc


def prep_weights(input_weight, hidden_weights, output_weights):
    """Stationaries: L1 fb0/fb1 + L2..L5 [128,128] each, L6 [128,96]."""
    hid_filter = np.kron(np.eye(4, dtype=np.float32), np.ones((8, 8), np.float32))
    out_filter = np.kron(np.eye(8, dtype=np.float32), np.ones((4, 3), np.float32))
    whm = hid_filter[None] * np.asarray(hidden_weights, np.float32)  # [4,32,32]
    wom = out_filter * np.asarray(output_weights, np.float32)  # [32,24]
    w_in = np.asarray(input_weight, np.float32)  # [64,32]

    mats = []
    for fb in range(2):
        mats.append(np.kron(np.eye(4, dtype=np.float32), w_in[32 * fb : 32 * fb + 32]))
    for l in range(4):
        mats.append(np.kron(np.eye(4, dtype=np.float32), whm[l]))
    mats.append(np.kron(np.eye(4, dtype=np.float32), wom))  # [128, 96]
    return np.concatenate(mats, axis=1)  # [128, 864]


def to_bf16(a):
    import ml_dtypes

    return np.asarray(a, np.float32).astype(ml_dtypes.bfloat16)


def prep_x(x, n_cores=N_CORES):
    """[B, 64] f32 -> [n_cores, 128, rows/2] bf16 feature-major group layout.

    xt[core, g*32+f, s*2048 + fb*1024 + j] = x[core*R + s*4096 + g*1024 + j,
                                              fb*32 + f]
    """
    x = to_bf16(x)
    b = x.shape[0]
    rows = b // n_cores
    n_slabs = rows // SLAB
    v = x.reshape(n_cores, n_slabs, 4, 1024, 2, 32)  # (core, s, g, j, fb, f)
    v = v.transpose(0, 2, 5, 1, 4, 3)  # (core, g, f, s, fb, j)
    return np.ascontiguousarray(v).reshape(n_cores, 128, n_slabs * 2048)


def decode_out(a, rows):
    """[96, rows/4] bf16 feature-major -> [rows, 24] f32.

    a[g*24 + c, s*1024 + j] = out[s*4096 + g*1024 + j, c]
    """
    n_slabs = rows // SLAB
    a = np.asarray(a)
    v = a.reshape(4, 24, n_slabs, 1024)  # (g, c, s, j)
    v = v.transpose(2, 0, 3, 1)  # (s, g, j, c)
    return np.ascontiguousarray(v).reshape(rows, 24).astype(np.float32)


def kernel(x, input_weight, hidden_weights, output_weights):
    from concourse.bass_utils import run_bass_kernel_spmd

    xt = prep_x(x)
    wbd = to_bf16(prep_weights(input_weight, hidden_weights, output_weights))

    rows = x.shape[0] // N_CORES
    nc = build_nc(rows)
    in_maps = [{"xt": xt[i], "wbd": wbd} for i in range(N_CORES)]
    res = run_bass_kernel_spmd(nc, in_maps, core_ids=list(range(N_CORES)))
    outs = [decode_out(res.results[i]["out"], rows) for i in range(N_CORES)]
    return np.concatenate(outs, axis=0)
